# revision 15
# baseline (speedup 1.0000x reference)
"""Multi-head attention (B=2,S=4096,E=768,H=12,D=64 + 16-token K/V prompt
prefix) on 8 Trainium2 NeuronCores.

Sharding: 2 batches x 4 head-groups (3 heads each). Each core computes QKV
projections for its 3 heads, full attention over its batch, and a partial
output projection (its 192 ctx channels); the host sums the 4 partials per
batch.

v2 design (vs the 485us baseline): the exp softmax work is split between
ScalarE (exact exp activation) and DVE (Schraudolph bit-trick exp: psum*A+B
-> int16 -> bitcast bf16), and the ctx matmuls run in the *natural*
orientation (out [128q, 65] per 128-k tile, charged only 65 rows by the
cost model instead of 512). ctx output is transposed back for the output
projection with xbar DMA transposes (free on all compute engines). All
matmuls bf16.

Per-core layout:
  qT[c,s], kT[c,s] = Wg @ x^T          (transposed, [128,2,S] head-pair tiles)
  v[s,c]           natural + ones col  (denominator accumulates in ctx col 64)
  scoresT[k,q]     = kT-tile^T @ qT    (psum [128, 2x512] = one kt-pair)
  expT             = Exp(scoresT/8)    (ScalarE exact | DVE Schraudolph)
  ctx[q,c+1]       = expT-tile^T @ v   (natural, psc [128, 12, 65] slices)
  ctx_norm         = ctx * recip(ctx[:,64]) per-partition scalars
  ctxT             = xbar DMA transpose of ctx_norm head-pairs
  outT[e,q]        = Wo-tiles^T @ ctxT (partial; host sums 4 groups, bf16)
"""

import sys
import threading

import numpy as np

if "/opt/trn_rl_repo" not in sys.path:
    sys.path.insert(0, "/opt/trn_rl_repo")

import ml_dtypes

BF16 = ml_dtypes.bfloat16

B, S, E, H, D, PP = 2, 4096, 768, 12, 64, 16
NCORES = 8
NG = 4          # head-groups (tensor parallel)
HL = H // NG    # 3 local heads
CL = HL * D     # 192 local channels
NKT = S // 128  # 32 k-tiles
NKP = NKT // 2  # 16 kt-pairs
SQB = 512       # q block width in the attention stream
NSQB = S // SQB
NST = S // 128  # v s-tiles
QT = 1024       # q width for projection blocks
NQB = S // QT   # 1024-q projection blocks
TRAIL = 8       # ctx trails scores/exp by this many slots
F_DVE = 0.30    # fraction of exp tiles computed on DVE via Schraudolph
LOG2E = 1.4426950408889634

_lock = threading.Lock()
_compiled = {}


def _build():
    import concourse.bass as bass  # noqa: F401
    import concourse.mybir as mybir
    import concourse.tile as tile
    from concourse import bacc

    f32 = mybir.dt.float32
    bf16 = mybir.dt.bfloat16
    i16 = mybir.dt.int16
    EXP = mybir.ActivationFunctionType.Exp
    MULT = mybir.AluOpType.mult
    ADD = mybir.AluOpType.add

    nc = bacc.Bacc("TRN2", target_bir_lowering=False, debug=False)

    xqT = nc.dram_tensor("xqT", [E, S], bf16, kind="ExternalInput").ap()
    xkT = nc.dram_tensor("xkT", [E, S], bf16, kind="ExternalInput").ap()
    xvT = nc.dram_tensor("xvT", [E, S], bf16, kind="ExternalInput").ap()
    wqT = nc.dram_tensor("wqT", [E, CL], bf16, kind="ExternalInput").ap()
    wkT = nc.dram_tensor("wkT", [E, CL], bf16, kind="ExternalInput").ap()
    wvT = nc.dram_tensor("wvT", [E, CL], bf16, kind="ExternalInput").ap()
    woT = nc.dram_tensor("woT", [CL, E], bf16, kind="ExternalInput").ap()
    bq = nc.dram_tensor("bq", [CL, 1], f32, kind="ExternalInput").ap()
    bk = nc.dram_tensor("bk", [CL, 1], f32, kind="ExternalInput").ap()
    bv = nc.dram_tensor("bv", [1, CL], f32, kind="ExternalInput").ap()
    kpT = nc.dram_tensor("kpT", [128, 2, PP], bf16, kind="ExternalInput").ap()
    vp = nc.dram_tensor("vp", [PP, HL, D + 1], bf16, kind="ExternalInput").ap()
    outT = nc.dram_tensor("outT", [E, S], bf16, kind="ExternalOutput").ap()

    # Schraudolph constants: psum holds raw q.k scores; exp arg = psum/8.
    # bf16 bits = 128*log2(e^(x)) + 16256 => psum * (128*log2e/8) + const.
    SCH_A = 128.0 * LOG2E * 0.125
    SCH_B = 16256.0 + 0.5 - 5.8  # +0.5 trunc->round, -5.8 sawtooth centering

    with tile.TileContext(nc) as tc:
        with tc.tile_pool(name="persist", bufs=1) as pers:
            # q-projection weights/bias first: they gate the very first
            # matmuls
            wq_sb = pers.tile([128, 6, CL], bf16)
            nc.gpsimd.dma_start(wq_sb[:], wqT.rearrange("(t p) c -> p t c", p=128))
            bq_sb = pers.tile([128, 2], f32)
            nc.gpsimd.dma_start(bq_sb[:, 0:1], bq[0:128, :])
            nc.gpsimd.dma_start(bq_sb[0:64, 1:2], bq[128:CL, :])

            wk_sb = pers.tile([128, 6, CL], bf16)
            wv_sb = pers.tile([128, 6, CL], bf16)
            wo_sb = pers.tile([128, 2, E], bf16)
            bk_sb = pers.tile([128, 2], f32)
            bvb_sb = pers.tile([128, CL], f32)
            kpT_sb = pers.tile([128, 2, PP], bf16)
            vp_sb = pers.tile([PP, HL, D + 1], bf16)

            # activations (all bf16)
            qT_sb = pers.tile([128, 2, S], bf16)
            kT_sb = pers.tile([128, 2, S], bf16)
            v_sb = pers.tile([128, NST, HL, D + 1], bf16)
            ctxT_sb = pers.tile([128, 2, S], bf16)
            # ctx_norm staging for xbar transposes: [qi, (h0,h1,h2,pad), d]
            ctxn_sb = pers.tile([128, SQB // 128, 4, D], bf16)

            nc.vector.memset(v_sb[:, :, :, D:D + 1], 1.0)
            nc.vector.memset(ctxn_sb[:, :, 3, :], 0.0)

            # ---------------- Phase 1a: Q / K projections ----------------
            with (
                tc.tile_pool(name="ps_proj", bufs=2, space="PSUM") as pp,
                tc.tile_pool(name="xq_pool", bufs=2) as xq_pool,
            ):
                def proj_block(xin, wsb, bsb, dst, sq):
                    p0 = pp.tile([128, QT], f32, tag="p0", name="p0")
                    p1 = pp.tile([64, QT], f32, tag="p1", name="p1")
                    xt = xq_pool.tile([128, 6, QT], bf16, tag="xt",
                                      name="xt")
                    nc.gpsimd.dma_start(
                        xt[:],
                        xin[:, sq * QT:(sq + 1) * QT].rearrange(
                            "(t p) c -> p t c", p=128),
                    )
                    for ech in range(6):
                        for n in range(QT // 512):
                            ns = slice(n * 512, (n + 1) * 512)
                            nc.tensor.matmul(
                                p0[:, ns], wsb[:, ech, 0:128],
                                xt[:, ech, ns],
                                start=(ech == 0), stop=(ech == 5),
                            )
                            nc.tensor.matmul(
                                p1[:, ns], wsb[:, ech, 128:CL],
                                xt[:, ech, ns],
                                start=(ech == 0), stop=(ech == 5),
                            )
                    ds = slice(sq * QT, (sq + 1) * QT)
                    nc.vector.tensor_scalar_add(
                        dst[:, 0, ds], p0[:], bsb[:, 0:1])
                    nc.vector.tensor_scalar_add(
                        dst[0:64, 1, ds], p1[:], bsb[0:64, 1:2])

                proj_block(xqT, wq_sb, bq_sb, qT_sb, 0)
                # stream remaining weights behind the critical q DMAs
                nc.gpsimd.dma_start(
                    wk_sb[:], wkT.rearrange("(t p) c -> p t c", p=128))
                nc.gpsimd.dma_start(bk_sb[:, 0:1], bk[0:128, :])
                nc.gpsimd.dma_start(bk_sb[0:64, 1:2], bk[128:CL, :])
                nc.gpsimd.dma_start(kpT_sb[:], kpT[:])
                nc.gpsimd.dma_start(
                    wv_sb[:], wvT.rearrange("(t p) c -> p t c", p=128))
                nc.gpsimd.dma_start(bvb_sb[:], bv.to_broadcast((128, CL)))
                nc.gpsimd.dma_start(vp_sb[:], vp[:])
                nc.gpsimd.dma_start(wo_sb[:, 0, :], woT[0:128, :])
                nc.gpsimd.dma_start(wo_sb[0:64, 1, :], woT[128:CL, :])

                for sq in range(NQB):
                    proj_block(xkT, wk_sb, bk_sb, kT_sb, sq)

            # ---------- attention stream ----------
            with (
                tc.tile_pool(name="ps_s", bufs=3, space="PSUM") as ps_s,
                tc.tile_pool(name="ps_acc", bufs=1, space="PSUM") as ps_acc,
                tc.tile_pool(name="ps_m", bufs=1, space="PSUM") as ps_m,
                tc.tile_pool(name="expt_pool", bufs=NKP + TRAIL + 3) as expt_pool,
                tc.tile_pool(name="expp_pool", bufs=2) as expp_pool,
                tc.tile_pool(name="rc_pool", bufs=4) as rc_pool,
                tc.tile_pool(name="xv_pool", bufs=2) as xv_pool,
                tc.tile_pool(name="xq2_pool", bufs=2) as xq2_pool,
                tc.tile_pool(name="out_pool", bufs=4) as out_pool,
            ):
                # ctx accumulator: one PSUM bank; slice (ql, h) = [128q, 65].
                # The sq-block's 4 q-subtiles are processed as two halves
                # (qi 0-1 streamed behind scores, qi 2-3 replayed per head
                # from retained exp tiles) so the accumulator fits one bank.
                # Interleaved accumulation chains share the bank, so matmul
                # start=True (bank-granular zeroing) cannot be used: slices
                # are DVE-memset between passes and every ctx matmul
                # accumulates with start=False.
                psc = ps_acc.tile([128, 2, HL, D + 1], f32, name="psc")
                nc.vector.memset(psc[:], 0.0)

                # ---- background q-projection for 1024-blocks 1..NQB-1 ----
                def make_bg_qproj(sq):
                    # each op is self-contained (psum tile allocated and
                    # evacuated within one emission) so the shared ps_m pool
                    # slots never stay held across stream slots
                    ops = []
                    state = {}

                    def dma_op():
                        xt2 = xq2_pool.tile([128, 6, QT], bf16, tag="xt2",
                                            name="xt2")
                        nc.gpsimd.dma_start(
                            xt2[:],
                            xqT[:, sq * QT:(sq + 1) * QT].rearrange(
                                "(t p) c -> p t c", p=128),
                        )
                        state["xt"] = xt2

                    ops.append(dma_op)

                    def mk_group(c, grp):
                        def op():
                            pt = ps_m.tile([128, 512], f32, tag="m",
                                           name="pq")
                            rows = 128 if grp == 0 else 64
                            wc = slice(0, 128) if grp == 0 else slice(128, CL)
                            for ech in range(6):
                                nc.tensor.matmul(
                                    pt[0:rows, :], wq_sb[:, ech, wc],
                                    state["xt"][:, ech,
                                                c * 512:(c + 1) * 512],
                                    start=(ech == 0), stop=(ech == 5),
                                )
                            qs = slice(sq * QT + c * 512,
                                       sq * QT + (c + 1) * 512)
                            if grp == 0:
                                nc.vector.tensor_scalar_add(
                                    qT_sb[:, 0, qs], pt[:, :], bq_sb[:, 0:1])
                            else:
                                nc.vector.tensor_scalar_add(
                                    qT_sb[0:64, 1, qs], pt[0:64, :],
                                    bq_sb[0:64, 1:2])
                        return op

                    for c in range(QT // 512):
                        for grp in range(2):
                            ops.append(mk_group(c, grp))
                    return ops

                SLOTS_PER_SQB = HL * NKP
                bg_work = []
                for sq in range(1, NQB):
                    t0 = max(4, 2 * sq * SLOTS_PER_SQB - 64)
                    for i, op in enumerate(make_bg_qproj(sq)):
                        bg_work.append((t0 + i, op))

                # ---- V projection (natural orientation) ----
                xvts = {}

                def load_xv(sqx):
                    xvt = xv_pool.tile([128, 6, QT], bf16, tag="xvt",
                                       name="xvt")
                    nc.gpsimd.dma_start(
                        xvt[:],
                        xvT[:, sqx * QT:(sqx + 1) * QT].rearrange(
                            "(t p) c -> p t c", p=128),
                    )
                    xvts[sqx] = xvt

                def emit_vproj(st):
                    sqx, stl = st // (QT // 128), st % (QT // 128)
                    if st == 0:
                        load_xv(0)
                    if stl == 0 and sqx + 1 < NQB:
                        load_xv(sqx + 1)
                    pv = ps_m.tile([128, 512], f32, tag="m", name="pv")
                    for ech in range(6):
                        nc.tensor.matmul(
                            pv[:, 0:CL],
                            xvts[sqx][:, ech, stl * 128:(stl + 1) * 128],
                            wv_sb[:, ech, :],
                            start=(ech == 0), stop=(ech == 5),
                        )
                    nc.vector.tensor_add(
                        v_sb[:, st, :, 0:D],
                        pv[:, 0:CL].rearrange("p (h d) -> p h d", h=HL),
                        bvb_sb[:].rearrange("p (h d) -> p h d", h=HL),
                    )
                    if stl == (QT // 128) - 1:
                        del xvts[sqx]

                # ---- prefix scores + exp for one sq-block (all 3 heads) ----
                expp_cur = [None]

                def emit_prefix(sqb):
                    psm = ps_m.tile([128, 512], f32, tag="m", name="psp")
                    qs = slice(sqb * SQB, (sqb + 1) * SQB)
                    ep = expp_pool.tile([PP, HL, SQB], bf16, tag="ep",
                                        name="ep")
                    for h in range(HL):
                        pr, po = h // 2, 64 * (h % 2)
                        nc.tensor.matmul(
                            psm[32 * h:32 * h + PP, :],
                            kpT_sb[po:po + 64, pr, :],
                            qT_sb[po:po + 64, pr, qs],
                            start=True, stop=True,
                        )
                        nc.scalar.activation(
                            ep[:, h, :], psm[32 * h:32 * h + PP, :],
                            EXP, scale=0.125)
                    expp_cur[0] = ep

                # ---- scores + exp for one (sqb, h, kt-pair) slot ----
                dve_acc = [0.0]

                def emit_scores_exp(sqb, h, kp):
                    pr, po = h // 2, 64 * (h % 2)
                    pss = ps_s.tile([128, 1024], f32, tag="pss", name="pss")
                    for j in range(2):
                        kt = 2 * kp + j
                        nc.tensor.matmul(
                            pss[:, j * 512:(j + 1) * 512],
                            kT_sb[po:po + 64, pr, kt * 128:(kt + 1) * 128],
                            qT_sb[po:po + 64, pr,
                                  sqb * SQB:(sqb + 1) * SQB],
                            start=True, stop=True,
                        )
                    expt = expt_pool.tile([128, 1024], bf16, tag="expt",
                                          name="expt")
                    dve_acc[0] += F_DVE
                    if dve_acc[0] >= 1.0:
                        dve_acc[0] -= 1.0
                        nc.vector.tensor_scalar(
                            expt[:].bitcast(i16), pss[:], SCH_A, SCH_B,
                            MULT, ADD)
                    else:
                        nc.scalar.activation(expt[:], pss[:], EXP, scale=0.125)
                    return expt

                # ---- ctx (natural orientation), one q-half at a time ----
                def emit_ctx_half(h, kp, expt, ep, half):
                    for j in range(2):
                        kt = 2 * kp + j
                        for ql in range(2):
                            qi = 2 * half + ql
                            acc = psc[:, ql, h, :]
                            if kp == 0 and j == 0:
                                nc.tensor.matmul(
                                    acc,
                                    ep[:, h, qi * 128:(qi + 1) * 128],
                                    vp_sb[:, h, :],
                                    start=False, stop=False,
                                    skip_group_check=True,
                                )
                            nc.tensor.matmul(
                                acc,
                                expt[:, j * 512 + qi * 128:
                                     j * 512 + (qi + 1) * 128],
                                v_sb[:, kt, h, :],
                                start=False, stop=False,
                                skip_group_check=True,
                            )

                # ---- norm + transpose + out-projection ----
                outproj_work = []

                def emit_norm_half(sqb, h, half):
                    for ql in range(2):
                        qi = 2 * half + ql
                        rc = rc_pool.tile([128, 1], f32, tag="rc", name="rc")
                        nc.vector.reciprocal(
                            rc[:], psc[:, ql, h, D:D + 1])
                        nc.vector.tensor_scalar_mul(
                            ctxn_sb[:, qi, h, :],
                            psc[:, ql, h, 0:D],
                            rc[:])
                    # re-zero this head's accumulator slices for the next
                    # pass (ordered after the norm reads by tile overlap)
                    nc.vector.memset(psc[:, :, h, :], 0.0)

                def finish_head(sqb, h, tiles, ep):
                    # qi 0-1 already streamed; norm them, then replay the
                    # retained exp tiles for qi 2-3, norm, then transposes
                    emit_norm_half(sqb, h, 0)
                    for kp, expt in tiles:
                        emit_ctx_half(h, kp, expt, ep, 1)
                    emit_norm_half(sqb, h, 1)
                    if h >= 1:
                        pr = 0 if h == 1 else 1
                        hs = slice(0, 2) if h == 1 else slice(2, 4)
                        for qi in range(SQB // 128):
                            qs = slice(sqb * SQB + qi * 128,
                                       sqb * SQB + (qi + 1) * 128)
                            nc.sync.dma_start(
                                ctxT_sb[:, pr, qs], ctxn_sb[:, qi, hs, :],
                                transpose=True)
                    if h == HL - 1:
                        for et in range(6):
                            outproj_work.append((et, sqb))

                def emit_outproj_tile(et, sqb):
                    es = slice(et * 128, (et + 1) * 128)
                    qs = slice(sqb * SQB, (sqb + 1) * SQB)
                    po3 = ps_m.tile([128, 512], f32, tag="m", name="po3")
                    nc.tensor.matmul(
                        po3[:], wo_sb[:, 0, es], ctxT_sb[:, 0, qs],
                        start=True, stop=False,
                    )
                    nc.tensor.matmul(
                        po3[:], wo_sb[0:64, 1, es], ctxT_sb[0:64, 1, qs],
                        start=False, stop=True,
                    )
                    ot = out_pool.tile([128, 512], bf16, tag="ot", name="ot")
                    nc.vector.tensor_copy(ot[:], po3[:])
                    nc.gpsimd.dma_start(outT[es, qs], ot[:])

                # ---- the slot stream ----
                slots = [(sqb, h, kp)
                         for sqb in range(NSQB)
                         for h in range(HL)
                         for kp in range(NKP)]
                pending = []
                head_tiles = []
                vst = 0

                def pop_one():
                    (s2, e2, ep2) = pending.pop(0)
                    sqb2, h2, kp2 = s2
                    emit_ctx_half(h2, kp2, e2, ep2, 0)
                    head_tiles.append((kp2, e2))
                    if kp2 == NKP - 1:
                        finish_head(sqb2, h2, head_tiles, ep2)
                        head_tiles.clear()

                for t, slot in enumerate(slots):
                    sqb, h, kp = slot
                    if h == 0 and kp == 0:
                        emit_prefix(sqb)
                    expt = emit_scores_exp(*slot)
                    pending.append((slot, expt, expp_cur[0]))
                    if vst < NST:
                        emit_vproj(vst)
                        vst += 1
                        if vst < NST:
                            emit_vproj(vst)
                            vst += 1
                    trail_eff = TRAIL if t < len(slots) - 20 else 2
                    for _ in range(3):
                        if len(pending) > trail_eff:
                            pop_one()
                        else:
                            break
                    while bg_work and bg_work[0][0] <= t:
                        bg_work.pop(0)[1]()
                    if outproj_work:
                        emit_outproj_tile(*outproj_work.pop(0))
                while pending:
                    pop_one()
                    if outproj_work:
                        emit_outproj_tile(*outproj_work.pop(0))
                for _, op in bg_work:
                    op()
                while outproj_work:
                    emit_outproj_tile(*outproj_work.pop(0))

    nc.compile()
    return nc


def _get_nc():
    with _lock:
        if "nc" not in _compiled:
            _compiled["nc"] = _build()
        return _compiled["nc"]


def _prep_in_maps(query, key, value, prompt, Wq, bq, Wk, bk, Wv, bv, Wo, bo):
    f32 = np.float32
    qT = [np.ascontiguousarray(query[b].T).astype(BF16) for b in range(B)]
    kT = [np.ascontiguousarray(key[b].T).astype(BF16) for b in range(B)]
    vT = [np.ascontiguousarray(value[b].T).astype(BF16) for b in range(B)]
    in_maps = []
    for core in range(NCORES):
        b, g = core // NG, core % NG
        cs = slice(g * CL, (g + 1) * CL)
        kp = np.zeros((128, 2, PP), BF16)
        vpa = np.zeros((PP, HL, D + 1), BF16)
        vpa[:, :, D] = 1.0
        for h in range(HL):
            gh = g * HL + h
            kp[64 * (h % 2):64 * (h % 2) + 64, h // 2, :] = (
                prompt[b, 0, :, gh, :].T.astype(BF16))
            vpa[:, h, 0:D] = prompt[b, 1, :, gh, :].astype(BF16)
        in_maps.append({
            "xqT": qT[b], "xkT": kT[b], "xvT": vT[b],
            "wqT": np.ascontiguousarray(Wq[cs, :].T).astype(BF16),
            "wkT": np.ascontiguousarray(Wk[cs, :].T).astype(BF16),
            "wvT": np.ascontiguousarray(Wv[cs, :].T).astype(BF16),
            "woT": np.ascontiguousarray(Wo[:, cs].T).astype(BF16),
            "bq": np.ascontiguousarray(bq[cs]).astype(f32).reshape(CL, 1),
            "bk": np.ascontiguousarray(bk[cs]).astype(f32).reshape(CL, 1),
            "bv": np.ascontiguousarray(bv[cs]).astype(f32).reshape(1, CL),
            "kpT": kp, "vp": vpa,
        })
    return in_maps


def _combine(results, bo):
    out = np.empty((B, S, E), np.float32)
    for b in range(B):
        acc = results[b * NG]["outT"].astype(np.float32)
        for g in range(1, NG):
            acc = acc + results[b * NG + g]["outT"].astype(np.float32)
        out[b] = acc.T
    if bo is not None and np.any(bo):
        out += np.asarray(bo, np.float32)
    return out


def run(inputs, trace=False):
    """Returns (output, exec_time_ns or None)."""
    from concourse import bass_utils

    nc = _get_nc()
    in_maps = _prep_in_maps(**{k: np.asarray(v) for k, v in inputs.items()})
    bo = np.asarray(inputs["bo"])
    res = bass_utils.run_bass_kernel_spmd(
        nc, in_maps, core_ids=list(range(NCORES)), trace=trace,
    )
    return _combine(res.results, bo), res.exec_time_ns


def kernel(**inputs):
    out, _ = run(inputs)
    return out
